# revision 13
# baseline (speedup 1.0000x reference)
"""Trainium2 Bass kernel for the nn_MultiHeadAttention problem.

Pure data-parallel over batch (B=8 -> 8 NeuronCores), no collectives.

The reference splits heads with a RAW reshape: (S, H*DK) -> (H, S, DK).
With S=1024, H=16, DK=64 that means, for projection output P2d[s, j]:
    head h   = s // 64
    pos  t   = (s % 64) * 16 + j // 64     (t in [0, 1024))
    depth d  = j % 64
We also use the "u" ordering u = (j//64)*64 + (s%64) for the k-side position
axis (contraction order is arbitrary as long as K-layout and V-layout agree).

Per-head attention machinery:
  scoresT (u_k, t_q) = K^T Q -> +penalty -> exp -> PV (V ones-augmented so
                               row 64 of the PV psum is the softmax denom Z)
  scores  (t_q, u_k) = Q^T K -> +penalty -> exp (accum_out = Z) -> attn out
                               (column permute u->t during the normalize)
"""

import numpy as np
from contextlib import ExitStack

import concourse.bass as bass
import concourse.bacc as bacc
import concourse.tile as tile
import concourse.mybir as mybir
from concourse.bass_utils import run_bass_kernel_spmd
from concourse.masks import make_identity

F32 = mybir.dt.float32
BF16 = mybir.dt.bfloat16
I32 = mybir.dt.int32
AF = mybir.ActivationFunctionType
OP = mybir.AluOpType

B, S, DM = 8, 1024, 1024
H, DK, DV = 16, 64, 64
HD = H * DK  # 1024
P = 128
NT = S // P  # 8
NKT = DM // P  # 8
G = DV + 1  # 65 cols per head in v_sb (64 V + 1 ones)
VW = H * G  # 1040 cols per u-tile in v_sb
N_CORES = 8

_CACHE = {}


def _build_nc():
    nc = bacc.Bacc("TRN2")

    d_q = nc.dram_tensor("x_q", [S, DM], F32, kind="ExternalInput")
    d_k = nc.dram_tensor("x_k", [S, DM], F32, kind="ExternalInput")
    d_v = nc.dram_tensor("x_v", [S, DM], F32, kind="ExternalInput")
    d_mask = nc.dram_tensor("mask", [S, S], I32, kind="ExternalInput")
    d_wq = nc.dram_tensor("Wq", [DM, HD], F32, kind="ExternalInput")
    d_wk = nc.dram_tensor("Wk", [DM, HD], F32, kind="ExternalInput")
    d_wv = nc.dram_tensor("Wv", [DM, HD], F32, kind="ExternalInput")
    d_wo = nc.dram_tensor("Wo", [HD, DM], F32, kind="ExternalInput")
    d_bq = nc.dram_tensor("bq", [1, HD], F32, kind="ExternalInput")
    d_bk = nc.dram_tensor("bk", [1, HD], F32, kind="ExternalInput")
    d_bv = nc.dram_tensor("bv", [1, HD], F32, kind="ExternalInput")
    d_bo = nc.dram_tensor("bo", [1, DM], F32, kind="ExternalInput")

    d_out = nc.dram_tensor("out", [S, DM], F32, kind="ExternalOutput")
    d_attn = nc.dram_tensor("attn", [H, S, S], F32, kind="ExternalOutput")

    with ExitStack() as ctx:
        tc = ctx.enter_context(tile.TileContext(nc))

        const = ctx.enter_context(tc.tile_pool(name="const", bufs=1))
        ident_f = const.tile([P, P], F32)
        make_identity(nc, ident_f)
        ident_b = const.tile([P, P], BF16)
        make_identity(nc, ident_b)
        ones_f = const.tile([1, 512], F32)
        nc.vector.memset(ones_f, 1.0)
        ones_b = const.tile([1, 512], BF16)
        nc.vector.memset(ones_b, 1.0)


        big = ctx.enter_context(tc.tile_pool(name="big", bufs=1))
        # per head h: [(h%2)*64 : +64] partitions, free block h//2
        qt_sb = big.tile([P, NT * S], BF16)  # Q heads: [d, t] (t-order free)
        kt_sb = big.tile([P, NT * S], BF16)  # K heads: [d, u] (u-order free)
        v_sb = big.tile([P, NT * VW], F32)  # V: [u, (head: 64 dv + 1 ones)]
        out2n_sb = big.tile([P, NT * S], F32)  # out2^T in j-partition layout

        nc.vector.memset(v_sb[:, :], 1.0)  # ones cols survive the V copies

        def in_transpose(xT, d_x, ppool, spool, wdt):
            """load x (s, dm); write xT (d-part, s-free) via PE transposes."""
            x_sb = spool.tile([P, NT * DM], F32, tag="xin")
            for st in range(NT):
                nc.sync.dma_start(
                    x_sb[:, st * DM : (st + 1) * DM], d_x[st * P : (st + 1) * P, :]
                )
            for dt_ in range(NKT):
                for half in range(2):
                    tp = ppool.tile([P, 512], F32, tag="tpin")
                    for q4 in range(4):
                        st = half * 4 + q4
                        nc.tensor.transpose(
                            tp[:, q4 * P : (q4 + 1) * P],
                            x_sb[:, st * DM + dt_ * P : st * DM + (dt_ + 1) * P],
                            ident_f,
                        )
                    nc.scalar.copy(
                        xT[:, dt_ * S + half * 512 : dt_ * S + (half + 1) * 512], tp
                    )

        # -------- Q/K projections (s-major form, then 64x64 block transposes) --
        def qk_projection(dst_sb, d_x, d_w, d_b, t_order):
            with tc.tile_pool(name="pxT", bufs=1) as pxT:
                xT = pxT.tile([P, NT * S], BF16, tag="xT")
                with (
                    tc.tile_pool(name="px", bufs=1) as px,
                    tc.tile_pool(name="ppsA", bufs=2, space="PSUM") as ppsA,
                ):
                    in_transpose(xT, d_x, ppsA, px, BF16)
                with (
                    tc.tile_pool(name="pw", bufs=1) as pw,
                    tc.tile_pool(name="pq2d", bufs=2) as pq2d,
                    tc.tile_pool(name="ppsB", bufs=2, space="PSUM") as ppsB,
                    tc.tile_pool(name="ppsT", bufs=2, space="PSUM") as ppsT,
                ):
                    b_row = pw.tile([1, HD], BF16, tag="brow")
                    nc.gpsimd.dma_start(out=b_row[:, :], in_=d_b[:, :])
                    w_sb = pw.tile([P, NKT * HD], BF16, tag="w")
                    for dt_ in range(NKT):
                        nc.gpsimd.dma_start(  # cast dma f32 -> bf16
                            out=w_sb[:, dt_ * HD : (dt_ + 1) * HD],
                            in_=d_w[dt_ * P : (dt_ + 1) * P, :],
                        )
                    for st in range(NT):
                        pp = ppsB.tile([P, S], F32, tag="pp")
                        for ch in range(2):
                            sl = slice(ch * 512, (ch + 1) * 512)
                            for dt_ in range(NKT):
                                nc.tensor.matmul(
                                    pp[:, sl],
                                    lhsT=xT[:, dt_ * S + st * P : dt_ * S + (st + 1) * P],
                                    rhs=w_sb[:, dt_ * HD + ch * 512 : dt_ * HD + (ch + 1) * 512],
                                    start=(dt_ == 0),
                                    stop=False,
                                )
                            nc.tensor.matmul(
                                pp[:, sl],
                                lhsT=ones_b[0:1, 0:P],
                                rhs=b_row[0:1, ch * 512 : (ch + 1) * 512],
                                start=False,
                                stop=True,
                            )
                        q2d = pq2d.tile([P, S], BF16, tag="q2d")
                        nc.scalar.copy(q2d[:, :], pp[:, :])
                        # 64x64 block transposes: (sl, d) -> (d, sl)
                        # transpose psum outputs must be partition-base 0;
                        # the ACT copy-out shifts e=1 back to partitions 64+.
                        for half in range(2):
                            for e in range(2):
                                eo = e * 64
                                tps = ppsT.tile([64, 512], BF16, tag="tps")
                                for s5 in range(8):
                                    cb = half * 8 + s5
                                    nc.tensor.transpose(
                                        tps[:, s5 * 64 : (s5 + 1) * 64],
                                        q2d[eo : eo + 64, cb * 64 : (cb + 1) * 64],
                                        ident_b[eo : eo + 64, eo : eo + 64],
                                    )
                                src = tps[:, :].rearrange("p (s5 d) -> p s5 d", s5=8)
                                if t_order:
                                    # col t = sl*16 + cb, cb = half*8 + s5
                                    dst = dst_sb[
                                        eo : eo + 64, st * S : (st + 1) * S
                                    ].rearrange("p (sl c) -> p c sl", c=16)[
                                        :, half * 8 : (half + 1) * 8, :
                                    ]
                                else:
                                    # u-order: col u = cb*64 + sl (dense)
                                    dst = dst_sb[
                                        eo : eo + 64,
                                        st * S + half * 512 : st * S + (half + 1) * 512,
                                    ].rearrange("p (s5 d) -> p s5 d", s5=8)
                                nc.scalar.copy(dst, src)

        qk_projection(qt_sb, d_q, d_wq, d_bq, t_order=True)
        qk_projection(kt_sb, d_k, d_wk, d_bk, t_order=False)

        # -------- V projection (j-major form, then 64x64 block transposes) ----
        with tc.tile_pool(name="pvt2d", bufs=1) as pvt2d:
            vt2d = pvt2d.tile([P, NT * S], F32, tag="vt2d")
            with tc.tile_pool(name="pxTv", bufs=1) as pxTv:
                xTv = pxTv.tile([P, NT * S], F32, tag="xTv")
                with (
                    tc.tile_pool(name="pxv", bufs=1) as pxv,
                    tc.tile_pool(name="ppsAv", bufs=2, space="PSUM") as ppsAv,
                ):
                    in_transpose(xTv, d_v, ppsAv, pxv, F32)
                with (
                    tc.tile_pool(name="pwv", bufs=18) as pwv,
                    tc.tile_pool(name="pbv", bufs=1) as pbv,
                    tc.tile_pool(name="ppsBv", bufs=2, space="PSUM") as ppsBv,
                ):
                    bv_row = pbv.tile([1, HD], F32, tag="bvrow")
                    nc.sync.dma_start(out=bv_row[:, :], in_=d_bv[:, :])
                    for jt3 in range(NT):
                        wts = []
                        for dt_ in range(NKT):
                            wt = pwv.tile([P, P], F32, tag="wv")
                            nc.sync.dma_start(
                                wt[:, :],
                                d_wv[dt_ * P : (dt_ + 1) * P, jt3 * P : (jt3 + 1) * P],
                            )
                            wts.append(wt)
                        pp = ppsBv.tile([P, S], F32, tag="ppv")
                        for ch in range(2):
                            sl = slice(ch * 512, (ch + 1) * 512)
                            for dt_ in range(NKT):
                                nc.tensor.matmul(
                                    pp[:, sl],
                                    lhsT=wts[dt_][:, :],
                                    rhs=xTv[:, dt_ * S + ch * 512 : dt_ * S + (ch + 1) * 512],
                                    start=(dt_ == 0),
                                    stop=False,
                                )
                            nc.tensor.matmul(
                                pp[:, sl],
                                lhsT=bv_row[0:1, jt3 * P : (jt3 + 1) * P],
                                rhs=ones_f[0:1, :],
                                start=False,
                                stop=True,
                            )
                        nc.scalar.copy(vt2d[:, jt3 * S : (jt3 + 1) * S], pp[:, :])
            # block transposes into v_sb: vt2d (d, s) blocks -> (sl, d)
            with tc.tile_pool(name="ppsTv", bufs=2, space="PSUM") as ppsTv:
                for h in range(H):
                    for half in range(2):
                        for par in range(2):
                            po = par * 64
                            tps = ppsTv.tile([64, 256], F32, tag="tpsv")
                            for c4 in range(4):
                                kt2 = half * 4 + c4
                                nc.tensor.transpose(
                                    tps[:, c4 * 64 : (c4 + 1) * 64],
                                    vt2d[po : po + 64, kt2 * S + 64 * h : kt2 * S + 64 * h + 64],
                                    ident_f[po : po + 64, po : po + 64],
                                )
                            src = tps[:, :].rearrange("p (c4 d) -> p c4 d", c4=4)
                            dst = v_sb[po : po + 64, :].rearrange(
                                "p (kt g) -> p kt g", kt=NT
                            )[:, half * 4 : (half + 1) * 4, h * G : h * G + DV]
                            nc.scalar.copy(dst, src)

        # ---------------- attention ----------------
        with ExitStack() as cctx:
            penp = cctx.enter_context(tc.tile_pool(name="pen", bufs=1))
            pen_sb = penp.tile([P, NT * S], BF16)  # (t_q part, u_k free)
            penT_sb = penp.tile([P, NT * S], BF16)  # (u_k part, t_q free)
            with (
                tc.tile_pool(name="mstream", bufs=3) as ms,
                tc.tile_pool(name="psC0", bufs=2, space="PSUM") as psC0,
            ):
                for qt in range(NT):
                    m_t = ms.tile([P, S], I32, tag="mask")
                    nc.sync.dma_start(m_t[:, :], d_mask[qt * P : (qt + 1) * P, :])
                    # penalty = mask*1920 - 1920, cols permuted t_k -> u_k
                    src = m_t[:, :].rearrange("p (sl c) -> p c sl", c=16)
                    dst = pen_sb[:, qt * S : (qt + 1) * S].rearrange(
                        "p (c sl) -> p c sl", c=16
                    )
                    nc.vector.tensor_scalar(
                        out=dst,
                        in0=src,
                        scalar1=1920.0,
                        scalar2=-1920.0,
                        op0=OP.mult,
                        op1=OP.add,
                    )
                for kt in range(NT):
                    for half in range(2):
                        tp = psC0.tile([P, 512], BF16, tag="ptp")
                        for q4 in range(4):
                            qt = half * 4 + q4
                            nc.tensor.transpose(
                                tp[:, q4 * P : (q4 + 1) * P],
                                pen_sb[:, qt * S + kt * P : qt * S + (kt + 1) * P],
                                ident_b,
                            )
                        nc.scalar.copy(
                            penT_sb[:, kt * S + half * 512 : kt * S + (half + 1) * 512],
                            tp[:, :],
                        )

            hp_expT = cctx.enter_context(tc.tile_pool(name="expT", bufs=3))
            hp_au = cctx.enter_context(tc.tile_pool(name="au", bufs=3))
            hp_af = cctx.enter_context(tc.tile_pool(name="af", bufs=3))
            hp_small = cctx.enter_context(tc.tile_pool(name="hsmall", bufs=8))
            hp_rzrow = cctx.enter_context(tc.tile_pool(name="rzrow", bufs=2))
            hp_rzb = cctx.enter_context(tc.tile_pool(name="rzb", bufs=2))
            psT = cctx.enter_context(tc.tile_pool(name="psT", bufs=2, space="PSUM"))
            psPV = cctx.enter_context(tc.tile_pool(name="psPV", bufs=1, space="PSUM"))
            psQ = cctx.enter_context(tc.tile_pool(name="psQ", bufs=1, space="PSUM"))

            for h in range(H):
                jt, po = h // 2, (h % 2) * 64

                # T path: scoresT (u_k, t_q) -> exp -> PV accumulate over u
                pv_ps = psPV.tile([G, S], F32, tag="pv")
                for kt in range(NT):
                    ps = psT.tile([P, S], F32, tag="scT")
                    for c2 in range(2):
                        sl = slice(c2 * 512, (c2 + 1) * 512)
                        nc.tensor.matmul(
                            ps[:, sl],
                            lhsT=kt_sb[po : po + 64, jt * S + kt * P : jt * S + (kt + 1) * P],
                            rhs=qt_sb[po : po + 64, jt * S + c2 * 512 : jt * S + (c2 + 1) * 512],
                            start=True,
                            stop=False,
                        )
                        nc.tensor.matmul(
                            ps[:, sl],
                            lhsT=ident_b,
                            rhs=penT_sb[:, kt * S + c2 * 512 : kt * S + (c2 + 1) * 512],
                            start=False,
                            stop=True,
                        )
                    et = hp_expT.tile([P, S], F32, tag="expT")
                    nc.scalar.activation(et[:, :], ps[:, :], AF.Exp, scale=1.0 / DK)
                    for c2 in range(2):
                        sl = slice(c2 * 512, (c2 + 1) * 512)
                        nc.tensor.matmul(
                            pv_ps[:, sl],
                            lhsT=v_sb[:, kt * VW + h * G : kt * VW + (h + 1) * G],
                            rhs=et[:, sl],
                            start=(kt == 0),
                            stop=(kt == NT - 1),
                        )
                # row G-1 of pv_ps is Z; normalize rows 0..64 and scatter:
                # psum[dv, t=(sl*16+cb3)] ->
                #   out2n_sb[(cb3%2)*64 + dv, (cb3//2)*S + 64h + sl]
                rz_row = hp_rzrow.tile([1, S], F32, tag="rzrow")
                nc.vector.reciprocal(rz_row[:, :], pv_ps[G - 1 : G, :])
                rzb = hp_rzb.tile([64, S], F32, tag="rzb")
                nc.gpsimd.partition_broadcast(rzb[:, :], rz_row[:, :])
                src_v = pv_ps[0:64, :].rearrange(
                    "p (sl c4 pr) -> p pr c4 sl", c4=8, pr=2
                )
                rz_v = rzb[:, :].rearrange("p (sl c4 pr) -> p pr c4 sl", c4=8, pr=2)
                for par in range(2):
                    dst = out2n_sb[par * 64 : (par + 1) * 64, :].rearrange(
                        "p (jt col) -> p jt col", jt=NT
                    )[:, :, 64 * h : 64 * h + 64]
                    nc.vector.tensor_tensor(
                        dst, src_v[:, par], rz_v[:, par], OP.mult
                    )

                # q path: scores (t_q, u_k) -> exp (accum Z) -> attn out
                for qt in range(NT):
                    ps2 = psQ.tile([P, S], F32, tag="scQ")
                    for c2 in range(2):
                        sl = slice(c2 * 512, (c2 + 1) * 512)
                        nc.tensor.matmul(
                            ps2[:, sl],
                            lhsT=qt_sb[po : po + 64, jt * S + qt * P : jt * S + (qt + 1) * P],
                            rhs=kt_sb[po : po + 64, jt * S + c2 * 512 : jt * S + (c2 + 1) * 512],
                            start=True,
                            stop=False,
                        )
                        nc.tensor.matmul(
                            ps2[:, sl],
                            lhsT=ident_b,
                            rhs=pen_sb[:, qt * S + c2 * 512 : qt * S + (c2 + 1) * 512],
                            start=False,
                            stop=True,
                        )
                    au = hp_au.tile([P, S], F32, tag="au")
                    zq = hp_small.tile([P, 1], F32, tag="zq")
                    nc.scalar.activation(
                        au[:, :], ps2[:, :], AF.Exp, scale=1.0 / DK, accum_out=zq[:, :]
                    )
                    rzq = hp_small.tile([P, 1], F32, tag="rzq")
                    nc.vector.reciprocal(rzq[:, :], zq[:, :])
                    # normalize + column permute u_k -> t_k
                    af = hp_af.tile([P, S], F32, tag="af")
                    src = au[:, :].rearrange("p (c sl) -> p c sl", c=16)
                    dst = af[:, :].rearrange("p (sl c) -> p c sl", c=16)
                    nc.vector.tensor_scalar_mul(dst, src, rzq[:, :])
                    nc.sync.dma_start(d_attn[h, qt * P : (qt + 1) * P, :], af[:, :])

        # ---------------- output projection ----------------
        with (
            tc.tile_pool(name="pwo", bufs=1) as pwo,
            tc.tile_pool(name="dstream", bufs=3) as ds_,
            tc.tile_pool(name="psD", bufs=2, space="PSUM") as psD,
        ):
            bo_bc = pwo.tile([P, DM], F32, tag="bobc")
            nc.gpsimd.dma_start(
                out=bo_bc[:, :],
                in_=bass.AP(tensor=d_bo, offset=0, ap=[[0, P], [1, DM]]),
            )
            wo_sb = pwo.tile([P, NKT * DM], F32)
            for jt2 in range(NKT):
                nc.sync.dma_start(
                    wo_sb[:, jt2 * DM : (jt2 + 1) * DM], d_wo[jt2 * P : (jt2 + 1) * P, :]
                )
            for qt in range(NT):
                for ec in range(2):
                    od_ps = psD.tile([P, 512], F32, tag="od")
                    for jt2 in range(NKT):
                        nc.tensor.matmul(
                            od_ps[:, :],
                            lhsT=out2n_sb[:, jt2 * S + qt * P : jt2 * S + (qt + 1) * P],
                            rhs=wo_sb[:, jt2 * DM + ec * 512 : jt2 * DM + (ec + 1) * 512],
                            start=(jt2 == 0),
                            stop=(jt2 == NKT - 1),
                        )
                    o_t = ds_.tile([P, 512], F32, tag="osb")
                    nc.vector.tensor_tensor(
                        o_t[:, :], od_ps[:, :], bo_bc[:, ec * 512 : (ec + 1) * 512], OP.add
                    )
                    nc.sync.dma_start(
                        d_out[qt * P : (qt + 1) * P, ec * 512 : (ec + 1) * 512], o_t[:, :]
                    )

    return nc


def get_nc(finalize=True):
    key = ("nc", finalize)
    if key not in _CACHE:
        nc = _build_nc()
        if finalize:
            nc.finalize()
        _CACHE[key] = nc
    return _CACHE[key]


def make_in_maps(inputs):
    q = np.asarray(inputs["query"], np.float32)
    k = np.asarray(inputs["key"], np.float32)
    v = np.asarray(inputs["value"], np.float32)
    m = np.asarray(inputs["mask"], np.int32)
    wq = np.asarray(inputs["Wq"], np.float32)
    wk = np.asarray(inputs["Wk"], np.float32)
    wv = np.asarray(inputs["Wv"], np.float32)
    wo = np.asarray(inputs["Wo"], np.float32)
    bq = np.asarray(inputs["bq"], np.float32).reshape(1, HD)
    bk = np.asarray(inputs["bk"], np.float32).reshape(1, HD)
    bv = np.asarray(inputs["bv"], np.float32).reshape(1, HD)
    bo = np.asarray(inputs["bo"], np.float32).reshape(1, DM)
    in_maps = []
    for c in range(N_CORES):
        in_maps.append(
            {
                "x_q": np.ascontiguousarray(q[c]),
                "x_k": np.ascontiguousarray(k[c]),
                "x_v": np.ascontiguousarray(v[c]),
                "mask": np.ascontiguousarray(m[c]),
                "Wq": wq,
                "Wk": wk,
                "Wv": wv,
                "Wo": wo,
                "bq": bq,
                "bk": bk,
                "bv": bv,
                "bo": bo,
            }
        )
    return in_maps


def kernel(**inputs):
    nc = get_nc(finalize=True)
    in_maps = make_in_maps(inputs)
    res = run_bass_kernel_spmd(nc, in_maps, core_ids=list(range(N_CORES)))
    out = np.stack([res.results[c]["out"] for c in range(N_CORES)], axis=0)
    attn = np.stack([res.results[c]["attn"] for c in range(N_CORES)], axis=0)
    return out, attn


# revision 24
# speedup vs baseline: 1.1181x; 1.1181x over previous
"""Trainium2 Bass kernel for the nn_MultiHeadAttention problem.

Pure data-parallel over batch (B=8 -> 8 NeuronCores), no collectives.

The reference splits heads with a RAW reshape: (S, H*DK) -> (H, S, DK).
With S=1024, H=16, DK=64 that means, for projection output P2d[s, j]:
    head h   = s // 64
    pos  t   = (s % 64) * 16 + j // 64     (t in [0, 1024))
    depth d  = j % 64
We also use the "u" ordering u = (j//64)*64 + (s%64) for the k-side position
axis (contraction order is arbitrary as long as K-layout and V-layout agree).

Per-head attention machinery:
  scoresT (u_k, t_q) = K^T Q -> +penalty -> exp -> PV (V ones-augmented so
                               row 64 of the PV psum is the softmax denom Z)
  scores  (t_q, u_k) = Q^T K -> +penalty -> exp (accum_out = Z) -> attn out
                               (column permute u->t during the normalize)
"""

import numpy as np
from contextlib import ExitStack

import concourse.bass as bass
import concourse.bacc as bacc
import concourse.tile as tile
import concourse.mybir as mybir
from concourse.bass_utils import run_bass_kernel_spmd
from concourse.masks import make_identity

F32 = mybir.dt.float32
BF16 = mybir.dt.bfloat16
I32 = mybir.dt.int32
AF = mybir.ActivationFunctionType
OP = mybir.AluOpType

B, S, DM = 8, 1024, 1024
H, DK, DV = 16, 64, 64
HD = H * DK  # 1024
P = 128
NT = S // P  # 8
NKT = DM // P  # 8
G = DV + 1  # 65 cols per head in v_sb (64 V + 1 ones)
VW = H * G  # 1040 cols per u-tile in v_sb
N_CORES = 8

_CACHE = {}


def _build_nc():
    nc = bacc.Bacc("TRN2")

    d_q = nc.dram_tensor("x_q", [S, DM], F32, kind="ExternalInput")
    d_k = nc.dram_tensor("x_k", [S, DM], F32, kind="ExternalInput")
    d_v = nc.dram_tensor("x_v", [S, DM], F32, kind="ExternalInput")
    d_mask = nc.dram_tensor("mask", [S, S], I32, kind="ExternalInput")
    d_wq = nc.dram_tensor("Wq", [DM, HD], F32, kind="ExternalInput")
    d_wk = nc.dram_tensor("Wk", [DM, HD], F32, kind="ExternalInput")
    d_wv = nc.dram_tensor("Wv", [DM, HD], F32, kind="ExternalInput")
    d_wo = nc.dram_tensor("Wo", [HD, DM], F32, kind="ExternalInput")
    d_bq = nc.dram_tensor("bq", [1, HD], F32, kind="ExternalInput")
    d_bk = nc.dram_tensor("bk", [1, HD], F32, kind="ExternalInput")
    d_bv = nc.dram_tensor("bv", [1, HD], F32, kind="ExternalInput")
    d_bo = nc.dram_tensor("bo", [1, DM], F32, kind="ExternalInput")

    d_out = nc.dram_tensor("out", [S, DM], F32, kind="ExternalOutput")
    d_attn = nc.dram_tensor("attn", [H, S, S], F32, kind="ExternalOutput")

    with ExitStack() as ctx:
        tc = ctx.enter_context(tile.TileContext(nc))

        const = ctx.enter_context(tc.tile_pool(name="const", bufs=1))
        ident_f = const.tile([P, P], F32)
        make_identity(nc, ident_f)
        ident_b = const.tile([P, P], BF16)
        make_identity(nc, ident_b)
        ones_f = const.tile([1, 512], F32)
        nc.vector.memset(ones_f, 1.0)
        ones_b = const.tile([1, 512], BF16)
        nc.vector.memset(ones_b, 1.0)


        big = ctx.enter_context(tc.tile_pool(name="big", bufs=1))
        # per head h: [(h%2)*64 : +64] partitions, free block h//2
        qt_sb = big.tile([P, NT * S], BF16)  # Q heads: [d, t] (t-order free)
        kt_sb = big.tile([P, NT * S], BF16)  # K heads: [d, u] (u-order free)
        v_sb = big.tile([P, NT * VW], F32)  # V: [u, (head: 64 dv + 1 ones)]
        out2n_sb = big.tile([P, NT * S], F32)  # out2^T in j-partition layout

        nc.vector.memset(v_sb[:, :], 1.0)  # ones cols survive the V copies

        def in_transpose(xT, d_x, ppool, spool, wdt):
            """load x (s, dm); write xT (d-part, s-free) via PE transposes."""
            x_sb = spool.tile([P, NT * DM], F32, tag="xin")
            for st in range(NT):
                nc.sync.dma_start(
                    x_sb[:, st * DM : (st + 1) * DM], d_x[st * P : (st + 1) * P, :]
                )
            for dt_ in range(NKT):
                for half in range(2):
                    tp = ppool.tile([P, 512], F32, tag="tpin")
                    for q4 in range(4):
                        st = half * 4 + q4
                        nc.tensor.transpose(
                            tp[:, q4 * P : (q4 + 1) * P],
                            x_sb[:, st * DM + dt_ * P : st * DM + (dt_ + 1) * P],
                            ident_f,
                        )
                    nc.vector.tensor_copy(
                        xT[:, dt_ * S + half * 512 : dt_ * S + (half + 1) * 512], tp
                    )

        # -------- Q/K projections (s-major form, then 64x64 block transposes) --
        def qk_projection(dst_sb, d_x, d_w, d_b):
            with tc.tile_pool(name="pxT", bufs=1) as pxT:
                xT = pxT.tile([P, NT * S], BF16, tag="xT")
                with (
                    tc.tile_pool(name="px", bufs=1) as px,
                    tc.tile_pool(name="ppsA", bufs=2, space="PSUM") as ppsA,
                ):
                    in_transpose(xT, d_x, ppsA, px, BF16)
                with (
                    tc.tile_pool(name="pw", bufs=1) as pw,
                    tc.tile_pool(name="pq2d", bufs=2) as pq2d,
                    tc.tile_pool(name="ppsB", bufs=2, space="PSUM") as ppsB,
                    tc.tile_pool(name="ppsT", bufs=2, space="PSUM") as ppsT,
                ):
                    b_row = pw.tile([1, HD], BF16, tag="brow")
                    nc.gpsimd.dma_start(out=b_row[:, :], in_=d_b[:, :])
                    w_sb = pw.tile([P, NKT * HD], BF16, tag="w")
                    for dt_ in range(NKT):
                        # stage f32 then DVE cast (round-to-nearest) to bf16
                        wstage = pq2d.tile([P, HD], F32, tag="wstage")
                        nc.sync.dma_start(
                            wstage[:, :], d_w[dt_ * P : (dt_ + 1) * P, :]
                        )
                        nc.vector.tensor_copy(
                            w_sb[:, dt_ * HD : (dt_ + 1) * HD], wstage[:, :]
                        )
                    for st in range(NT):
                        pp = ppsB.tile([P, S], F32, tag="pp")
                        for ch in range(2):
                            sl = slice(ch * 512, (ch + 1) * 512)
                            for dt_ in range(NKT):
                                nc.tensor.matmul(
                                    pp[:, sl],
                                    lhsT=xT[:, dt_ * S + st * P : dt_ * S + (st + 1) * P],
                                    rhs=w_sb[:, dt_ * HD + ch * 512 : dt_ * HD + (ch + 1) * 512],
                                    start=(dt_ == 0),
                                    stop=False,
                                )
                            nc.tensor.matmul(
                                pp[:, sl],
                                lhsT=ones_b[0:1, 0:P],
                                rhs=b_row[0:1, ch * 512 : (ch + 1) * 512],
                                start=False,
                                stop=True,
                            )
                        q2d = pq2d.tile([P, S], BF16, tag="q2d")
                        nc.scalar.copy(q2d[:, :], pp[:, :])
                        # 64x64 block transposes: (sl, d) -> (d, sl)
                        # transpose psum outputs must be partition-base 0;
                        # the ACT copy-out shifts e=1 back to partitions 64+.
                        for half in range(2):
                            for e in range(2):
                                eo = e * 64
                                tps = ppsT.tile([64, 512], BF16, tag="tps")
                                for s5 in range(8):
                                    cb = half * 8 + s5
                                    nc.tensor.transpose(
                                        tps[:, s5 * 64 : (s5 + 1) * 64],
                                        q2d[eo : eo + 64, cb * 64 : (cb + 1) * 64],
                                        ident_b[eo : eo + 64, eo : eo + 64],
                                    )
                                # u-order store: col u = cb*64 + sl (dense)
                                nc.scalar.copy(
                                    dst_sb[
                                        eo : eo + 64,
                                        st * S + half * 512 : st * S + (half + 1) * 512,
                                    ],
                                    tps[:, :],
                                )

        qk_projection(qt_sb, d_q, d_wq, d_bq)
        qk_projection(kt_sb, d_k, d_wk, d_bk)

        # -------- V projection (j-major form, then 64x64 block transposes) ----
        with tc.tile_pool(name="pvt2d", bufs=1) as pvt2d:
            vt2d = pvt2d.tile([P, NT * S], F32, tag="vt2d")
            with tc.tile_pool(name="pxTv", bufs=1) as pxTv:
                xTv = pxTv.tile([P, NT * S], F32, tag="xTv")
                with (
                    tc.tile_pool(name="pxv", bufs=1) as pxv,
                    tc.tile_pool(name="ppsAv", bufs=2, space="PSUM") as ppsAv,
                ):
                    in_transpose(xTv, d_v, ppsAv, pxv, F32)
                with (
                    tc.tile_pool(name="pwv", bufs=18) as pwv,
                    tc.tile_pool(name="pbv", bufs=1) as pbv,
                    tc.tile_pool(name="ppsBv", bufs=2, space="PSUM") as ppsBv,
                ):
                    bv_row = pbv.tile([1, HD], F32, tag="bvrow")
                    nc.sync.dma_start(out=bv_row[:, :], in_=d_bv[:, :])
                    for jt3 in range(NT):
                        wts = []
                        for dt_ in range(NKT):
                            wt = pwv.tile([P, P], F32, tag="wv")
                            nc.sync.dma_start(
                                wt[:, :],
                                d_wv[dt_ * P : (dt_ + 1) * P, jt3 * P : (jt3 + 1) * P],
                            )
                            wts.append(wt)
                        pp = ppsBv.tile([P, S], F32, tag="ppv")
                        for ch in range(2):
                            sl = slice(ch * 512, (ch + 1) * 512)
                            for dt_ in range(NKT):
                                nc.tensor.matmul(
                                    pp[:, sl],
                                    lhsT=wts[dt_][:, :],
                                    rhs=xTv[:, dt_ * S + ch * 512 : dt_ * S + (ch + 1) * 512],
                                    start=(dt_ == 0),
                                    stop=False,
                                )
                            nc.tensor.matmul(
                                pp[:, sl],
                                lhsT=bv_row[0:1, jt3 * P : (jt3 + 1) * P],
                                rhs=ones_f[0:1, :],
                                start=False,
                                stop=True,
                            )
                        nc.scalar.copy(vt2d[:, jt3 * S : (jt3 + 1) * S], pp[:, :])
            # block transposes into v_sb: vt2d (d, s) blocks -> (sl, d)
            with tc.tile_pool(name="ppsTv", bufs=2, space="PSUM") as ppsTv:
                for h in range(H):
                    for half in range(2):
                        for par in range(2):
                            po = par * 64
                            tps = ppsTv.tile([64, 256], F32, tag="tpsv")
                            for c4 in range(4):
                                kt2 = half * 4 + c4
                                nc.tensor.transpose(
                                    tps[:, c4 * 64 : (c4 + 1) * 64],
                                    vt2d[po : po + 64, kt2 * S + 64 * h : kt2 * S + 64 * h + 64],
                                    ident_f[po : po + 64, po : po + 64],
                                )
                            src = tps[:, :].rearrange("p (c4 d) -> p c4 d", c4=4)
                            dst = v_sb[po : po + 64, :].rearrange(
                                "p (kt g) -> p kt g", kt=NT
                            )[:, half * 4 : (half + 1) * 4, h * G : h * G + DV]
                            nc.scalar.copy(dst, src)

        # ---------------- attention ----------------
        with ExitStack() as cctx:
            penp = cctx.enter_context(tc.tile_pool(name="pen", bufs=1))
            maskf_sb = penp.tile([P, NT * S], BF16)  # 0/1 mask (u_q part, u_k free)
            penT_sb = penp.tile([P, NT * S], BF16)  # penalty^T (u_k part, u_q free)
            with (
                tc.tile_pool(name="mstream", bufs=3) as ms,
                tc.tile_pool(name="psC0", bufs=2, space="PSUM") as psC0,
            ):
                # mask rows are loaded u-permuted (row t(uq) -> partition uq)
                mask_uq = d_mask[:, :].rearrange("(sl c) k -> c sl k", sl=64)
                for qt in range(NT):
                    m_t = ms.tile([P, S], I32, tag="mask")
                    for cpar in range(2):
                        nc.sync.dma_start(
                            m_t[cpar * 64 : (cpar + 1) * 64, :],
                            mask_uq[2 * qt + cpar],
                        )
                    # 0/1 mask cast to bf16, cols permuted t_k -> u_k
                    src = m_t[:, :].rearrange("p (sl c) -> p c sl", c=16)
                    dst = maskf_sb[:, qt * S : (qt + 1) * S].rearrange(
                        "p (c sl) -> p c sl", c=16
                    )
                    nc.vector.tensor_copy(dst, src)
                for kt in range(NT):
                    for half in range(2):
                        tp = psC0.tile([P, 512], BF16, tag="ptp")
                        for q4 in range(4):
                            qt = half * 4 + q4
                            nc.tensor.transpose(
                                tp[:, q4 * P : (q4 + 1) * P],
                                maskf_sb[:, qt * S + kt * P : qt * S + (kt + 1) * P],
                                ident_b,
                            )
                        # penalty^T = mask^T * 1920 - 1920  (0 kept, -1920 masked)
                        nc.vector.tensor_scalar(
                            out=penT_sb[
                                :, kt * S + half * 512 : kt * S + (half + 1) * 512
                            ],
                            in0=tp[:, :],
                            scalar1=1920.0,
                            scalar2=-1920.0,
                            op0=OP.mult,
                            op1=OP.add,
                        )

            hp_expT = cctx.enter_context(tc.tile_pool(name="expT", bufs=3))
            hp_au = cctx.enter_context(tc.tile_pool(name="au", bufs=3))
            hp_af = cctx.enter_context(tc.tile_pool(name="af", bufs=3))
            hp_small = cctx.enter_context(tc.tile_pool(name="hsmall", bufs=4))
            hp_rzb = cctx.enter_context(tc.tile_pool(name="rzb", bufs=2))
            psT = cctx.enter_context(tc.tile_pool(name="psT", bufs=2, space="PSUM"))
            psPV = cctx.enter_context(tc.tile_pool(name="psPV", bufs=1, space="PSUM"))
            psQ = cctx.enter_context(tc.tile_pool(name="psQ", bufs=3, space="PSUM"))
            psR = cctx.enter_context(tc.tile_pool(name="psR", bufs=1, space="PSUM"))

            for h in range(H):
                jt, po = h // 2, (h % 2) * 64

                # T path: scoresT (u_k, u_q) -> exp -> PV accumulate over u_k
                pv_ps = psPV.tile([G, S], F32, tag="pv")
                for kt in range(NT):
                    et = hp_expT.tile([P, S], F32, tag="expT")
                    for c2 in range(2):
                        sl = slice(c2 * 512, (c2 + 1) * 512)
                        ps = psT.tile([P, 512], F32, tag="scT")
                        nc.tensor.matmul(
                            ps[:, :],
                            lhsT=kt_sb[po : po + 64, jt * S + kt * P : jt * S + (kt + 1) * P],
                            rhs=qt_sb[po : po + 64, jt * S + c2 * 512 : jt * S + (c2 + 1) * 512],
                            start=True,
                            stop=False,
                        )
                        nc.tensor.matmul(
                            ps[:, :],
                            lhsT=ident_b,
                            rhs=penT_sb[:, kt * S + c2 * 512 : kt * S + (c2 + 1) * 512],
                            start=False,
                            stop=True,
                        )
                        nc.scalar.activation(et[:, sl], ps[:, :], AF.Exp, scale=1.0 / DK)
                        nc.tensor.matmul(
                            pv_ps[:, sl],
                            lhsT=v_sb[:, kt * VW + h * G : kt * VW + (h + 1) * G],
                            rhs=et[:, sl],
                            start=(kt == 0),
                            stop=(kt == NT - 1),
                        )
                # row G-1 of pv_ps is Z.  1/Z via ACT: rzb = exp(-ln(Z)),
                # broadcast to 64 partitions for the out2 normalize.
                lnz = hp_small.tile([1, S], F32, tag="lnz")
                nc.scalar.activation(lnz[:, :], pv_ps[G - 1 : G, :], AF.Ln)
                lnzb = hp_rzb.tile([64, S], F32, tag="lnzb")
                nc.gpsimd.partition_broadcast(lnzb[:, :], lnz[:, :])
                rzb = hp_rzb.tile([64, S], F32, tag="rzb")
                nc.scalar.activation(rzb[:, :], lnzb[:, :], AF.Exp, scale=-1.0)
                # normalize rows 0..64 and scatter into out2n:
                # psum[dv, u=(cb*64+sl)] ->
                #   out2n_sb[(cb%2)*64 + dv, (cb//2)*S + 64h + sl]
                src_v = pv_ps[0:64, :].rearrange(
                    "p (c4 pr sl) -> p pr c4 sl", c4=8, pr=2
                )
                rz_v = rzb[:, :].rearrange("p (c4 pr sl) -> p pr c4 sl", c4=8, pr=2)
                for par in range(2):
                    dst = out2n_sb[par * 64 : (par + 1) * 64, :].rearrange(
                        "p (jt col) -> p jt col", jt=NT
                    )[:, :, 64 * h : 64 * h + 64]
                    nc.vector.tensor_tensor(
                        dst, src_v[:, par], rz_v[:, par], OP.mult
                    )
                # 1/Z transposed to q-layout: (128 uq, 1) per q-tile, via PE
                rzt_ps = psR.tile([P, 512], F32, tag="rzt")
                for qt in range(NT):
                    nc.tensor.transpose(
                        rzt_ps[:, qt * 64 : (qt + 1) * 64],
                        rzb[0:64, qt * P : (qt + 1) * P],
                        ident_f[0:64, 0:64],
                    )
                rzt8 = hp_small.tile([P, NT], F32, tag="rzt8")
                nc.scalar.copy(
                    rzt8[:, :],
                    rzt_ps[:, :].rearrange("p (q o) -> p q o", q=NT)[:, :, 0],
                )

                # q path: scores (u_q, u_k) -> exp -> fused mask+normalize
                for qt in range(NT):
                    au = hp_au.tile([P, S], F32, tag="au")
                    for c2 in range(2):
                        sl = slice(c2 * 512, (c2 + 1) * 512)
                        ps2 = psQ.tile([P, 512], F32, tag="scQ")
                        nc.tensor.matmul(
                            ps2[:, :],
                            lhsT=qt_sb[po : po + 64, jt * S + qt * P : jt * S + (qt + 1) * P],
                            rhs=kt_sb[po : po + 64, jt * S + c2 * 512 : jt * S + (c2 + 1) * 512],
                            start=True,
                            stop=True,
                        )
                        nc.scalar.activation(au[:, sl], ps2[:, :], AF.Exp, scale=1.0 / DK)
                    # af = (au * 1/Z) * mask, with column permute u_k -> t_k
                    af = hp_af.tile([P, S], F32, tag="af")
                    src = au[:, :].rearrange("p (c sl) -> p c sl", c=16)
                    msrc = maskf_sb[:, qt * S : (qt + 1) * S].rearrange(
                        "p (c sl) -> p c sl", c=16
                    )
                    dst = af[:, :].rearrange("p (sl c) -> p c sl", c=16)
                    nc.vector.scalar_tensor_tensor(
                        dst, src, rzt8[:, qt : qt + 1], msrc, OP.mult, OP.mult
                    )
                    attn_uq = d_attn[h, :, :].rearrange("(sl c) k -> c sl k", sl=64)
                    for cpar in range(2):
                        nc.sync.dma_start(
                            attn_uq[2 * qt + cpar],
                            af[cpar * 64 : (cpar + 1) * 64, :],
                        )

        # ---------------- output projection ----------------
        with (
            tc.tile_pool(name="pwo", bufs=1) as pwo,
            tc.tile_pool(name="dstream", bufs=3) as ds_,
            tc.tile_pool(name="psD", bufs=2, space="PSUM") as psD,
        ):
            bo_bc = pwo.tile([P, DM], F32, tag="bobc")
            nc.gpsimd.dma_start(
                out=bo_bc[:, :],
                in_=bass.AP(tensor=d_bo, offset=0, ap=[[0, P], [1, DM]]),
            )
            wo_sb = pwo.tile([P, NKT * DM], F32)
            for jt2 in range(NKT):
                nc.sync.dma_start(
                    wo_sb[:, jt2 * DM : (jt2 + 1) * DM], d_wo[jt2 * P : (jt2 + 1) * P, :]
                )
            for qt in range(NT):
                for ec in range(2):
                    od_ps = psD.tile([P, 512], F32, tag="od")
                    for jt2 in range(NKT):
                        nc.tensor.matmul(
                            od_ps[:, :],
                            lhsT=out2n_sb[:, jt2 * S + qt * P : jt2 * S + (qt + 1) * P],
                            rhs=wo_sb[:, jt2 * DM + ec * 512 : jt2 * DM + (ec + 1) * 512],
                            start=(jt2 == 0),
                            stop=(jt2 == NKT - 1),
                        )
                    o_t = ds_.tile([P, 512], F32, tag="osb")
                    nc.vector.tensor_tensor(
                        o_t[:, :], od_ps[:, :], bo_bc[:, ec * 512 : (ec + 1) * 512], OP.add
                    )
                    nc.sync.dma_start(
                        d_out[qt * P : (qt + 1) * P, ec * 512 : (ec + 1) * 512], o_t[:, :]
                    )

    return nc


def get_nc(finalize=True):
    key = ("nc", finalize)
    if key not in _CACHE:
        nc = _build_nc()
        if finalize:
            nc.finalize()
        _CACHE[key] = nc
    return _CACHE[key]


def make_in_maps(inputs):
    q = np.asarray(inputs["query"], np.float32)
    k = np.asarray(inputs["key"], np.float32)
    v = np.asarray(inputs["value"], np.float32)
    m = np.asarray(inputs["mask"], np.int32)
    wq = np.asarray(inputs["Wq"], np.float32)
    wk = np.asarray(inputs["Wk"], np.float32)
    wv = np.asarray(inputs["Wv"], np.float32)
    wo = np.asarray(inputs["Wo"], np.float32)
    bq = np.asarray(inputs["bq"], np.float32).reshape(1, HD)
    bk = np.asarray(inputs["bk"], np.float32).reshape(1, HD)
    bv = np.asarray(inputs["bv"], np.float32).reshape(1, HD)
    bo = np.asarray(inputs["bo"], np.float32).reshape(1, DM)
    in_maps = []
    for c in range(N_CORES):
        in_maps.append(
            {
                "x_q": np.ascontiguousarray(q[c]),
                "x_k": np.ascontiguousarray(k[c]),
                "x_v": np.ascontiguousarray(v[c]),
                "mask": np.ascontiguousarray(m[c]),
                "Wq": wq,
                "Wk": wk,
                "Wv": wv,
                "Wo": wo,
                "bq": bq,
                "bk": bk,
                "bv": bv,
                "bo": bo,
            }
        )
    return in_maps


def kernel(**inputs):
    nc = get_nc(finalize=True)
    in_maps = make_in_maps(inputs)
    res = run_bass_kernel_spmd(nc, in_maps, core_ids=list(range(N_CORES)))
    out = np.stack([res.results[c]["out"] for c in range(N_CORES)], axis=0)
    attn = np.stack([res.results[c]["attn"] for c in range(N_CORES)], axis=0)
    return out, attn


# revision 26
# speedup vs baseline: 1.1197x; 1.0014x over previous
"""Trainium2 Bass kernel for the nn_MultiHeadAttention problem.

Pure data-parallel over batch (B=8 -> 8 NeuronCores), no collectives.

The reference splits heads with a RAW reshape: (S, H*DK) -> (H, S, DK).
With S=1024, H=16, DK=64 that means, for projection output P2d[s, j]:
    head h   = s // 64
    pos  t   = (s % 64) * 16 + j // 64     (t in [0, 1024))
    depth d  = j % 64
We also use the "u" ordering u = (j//64)*64 + (s%64) for the k-side position
axis (contraction order is arbitrary as long as K-layout and V-layout agree).

Per-head attention machinery:
  scoresT (u_k, t_q) = K^T Q -> +penalty -> exp -> PV (V ones-augmented so
                               row 64 of the PV psum is the softmax denom Z)
  scores  (t_q, u_k) = Q^T K -> +penalty -> exp (accum_out = Z) -> attn out
                               (column permute u->t during the normalize)
"""

import numpy as np
from contextlib import ExitStack

import concourse.bass as bass
import concourse.bacc as bacc
import concourse.tile as tile
import concourse.mybir as mybir
from concourse.bass_utils import run_bass_kernel_spmd
from concourse.masks import make_identity

F32 = mybir.dt.float32
BF16 = mybir.dt.bfloat16
I32 = mybir.dt.int32
AF = mybir.ActivationFunctionType
OP = mybir.AluOpType

B, S, DM = 8, 1024, 1024
H, DK, DV = 16, 64, 64
HD = H * DK  # 1024
P = 128
NT = S // P  # 8
NKT = DM // P  # 8
G = DV + 1  # 65 cols per head in v_sb (64 V + 1 ones)
VW = H * G  # 1040 cols per u-tile in v_sb
N_CORES = 8

_CACHE = {}


def _build_nc():
    nc = bacc.Bacc("TRN2")

    d_q = nc.dram_tensor("x_q", [S, DM], F32, kind="ExternalInput")
    d_k = nc.dram_tensor("x_k", [S, DM], F32, kind="ExternalInput")
    d_v = nc.dram_tensor("x_v", [S, DM], F32, kind="ExternalInput")
    d_mask = nc.dram_tensor("mask", [S, S], I32, kind="ExternalInput")
    d_wq = nc.dram_tensor("Wq", [DM, HD], F32, kind="ExternalInput")
    d_wk = nc.dram_tensor("Wk", [DM, HD], F32, kind="ExternalInput")
    d_wv = nc.dram_tensor("Wv", [DM, HD], F32, kind="ExternalInput")
    d_wo = nc.dram_tensor("Wo", [HD, DM], F32, kind="ExternalInput")
    d_bq = nc.dram_tensor("bq", [1, HD], F32, kind="ExternalInput")
    d_bk = nc.dram_tensor("bk", [1, HD], F32, kind="ExternalInput")
    d_bv = nc.dram_tensor("bv", [1, HD], F32, kind="ExternalInput")
    d_bo = nc.dram_tensor("bo", [1, DM], F32, kind="ExternalInput")

    d_out = nc.dram_tensor("out", [S, DM], F32, kind="ExternalOutput")
    d_attn = nc.dram_tensor("attn", [H, S, S], F32, kind="ExternalOutput")

    with ExitStack() as ctx:
        tc = ctx.enter_context(tile.TileContext(nc))

        const = ctx.enter_context(tc.tile_pool(name="const", bufs=1))
        ident_f = const.tile([P, P], F32)
        make_identity(nc, ident_f)
        ident_b = const.tile([P, P], BF16)
        make_identity(nc, ident_b)
        ones_f = const.tile([1, 512], F32)
        nc.vector.memset(ones_f, 1.0)
        ones_b = const.tile([1, 512], BF16)
        nc.vector.memset(ones_b, 1.0)


        big = ctx.enter_context(tc.tile_pool(name="big", bufs=1))
        # per head h: [(h%2)*64 : +64] partitions, free block h//2
        qt_sb = big.tile([P, NT * S], BF16)  # Q heads: [d, t] (t-order free)
        kt_sb = big.tile([P, NT * S], BF16)  # K heads: [d, u] (u-order free)
        v_sb = big.tile([P, NT * VW], F32)  # V: [u, (head: 64 dv + 1 ones)]
        out2n_sb = big.tile([P, NT * S], F32)  # out2^T in j-partition layout

        nc.vector.memset(v_sb[:, :], 1.0)  # ones cols survive the V copies

        def in_transpose(xT, d_x, ppool, spool, wdt):
            """load x (s, dm); write xT (d-part, s-free) via PE transposes."""
            x_sb = spool.tile([P, NT * DM], F32, tag="xin")
            for st in range(NT):
                nc.sync.dma_start(
                    x_sb[:, st * DM : (st + 1) * DM], d_x[st * P : (st + 1) * P, :]
                )
            for dt_ in range(NKT):
                for half in range(2):
                    tp = ppool.tile([P, 512], F32, tag="tpin")
                    for q4 in range(4):
                        st = half * 4 + q4
                        nc.tensor.transpose(
                            tp[:, q4 * P : (q4 + 1) * P],
                            x_sb[:, st * DM + dt_ * P : st * DM + (dt_ + 1) * P],
                            ident_f,
                        )
                    nc.vector.tensor_copy(
                        xT[:, dt_ * S + half * 512 : dt_ * S + (half + 1) * 512], tp
                    )

        # -------- Q/K projections (s-major form, then 64x64 block transposes) --
        def qk_projection(dst_sb, d_x, d_w, d_b):
            with tc.tile_pool(name="pxT", bufs=1) as pxT:
                xT = pxT.tile([P, NT * S], BF16, tag="xT")
                with (
                    tc.tile_pool(name="px", bufs=1) as px,
                    tc.tile_pool(name="ppsA", bufs=2, space="PSUM") as ppsA,
                ):
                    in_transpose(xT, d_x, ppsA, px, BF16)
                with (
                    tc.tile_pool(name="pw", bufs=1) as pw,
                    tc.tile_pool(name="pq2d", bufs=2) as pq2d,
                    tc.tile_pool(name="ppsB", bufs=2, space="PSUM") as ppsB,
                    tc.tile_pool(name="ppsT", bufs=2, space="PSUM") as ppsT,
                ):
                    b_row = pw.tile([1, HD], BF16, tag="brow")
                    nc.gpsimd.dma_start(out=b_row[:, :], in_=d_b[:, :])
                    w_sb = pw.tile([P, NKT * HD], BF16, tag="w")
                    for dt_ in range(NKT):
                        # stage f32 then DVE cast (round-to-nearest) to bf16
                        wstage = pq2d.tile([P, HD], F32, tag="wstage")
                        nc.sync.dma_start(
                            wstage[:, :], d_w[dt_ * P : (dt_ + 1) * P, :]
                        )
                        nc.vector.tensor_copy(
                            w_sb[:, dt_ * HD : (dt_ + 1) * HD], wstage[:, :]
                        )
                    for st in range(NT):
                        pp = ppsB.tile([P, S], F32, tag="pp")
                        for ch in range(2):
                            sl = slice(ch * 512, (ch + 1) * 512)
                            for dt_ in range(NKT):
                                nc.tensor.matmul(
                                    pp[:, sl],
                                    lhsT=xT[:, dt_ * S + st * P : dt_ * S + (st + 1) * P],
                                    rhs=w_sb[:, dt_ * HD + ch * 512 : dt_ * HD + (ch + 1) * 512],
                                    start=(dt_ == 0),
                                    stop=False,
                                )
                            nc.tensor.matmul(
                                pp[:, sl],
                                lhsT=ones_b[0:1, 0:P],
                                rhs=b_row[0:1, ch * 512 : (ch + 1) * 512],
                                start=False,
                                stop=True,
                            )
                        q2d = pq2d.tile([P, S], BF16, tag="q2d")
                        nc.scalar.copy(q2d[:, :], pp[:, :])
                        # 64x64 block transposes: (sl, d) -> (d, sl)
                        # transpose psum outputs must be partition-base 0;
                        # the ACT copy-out shifts e=1 back to partitions 64+.
                        for half in range(2):
                            for e in range(2):
                                eo = e * 64
                                tps = ppsT.tile([64, 512], BF16, tag="tps")
                                for s5 in range(8):
                                    cb = half * 8 + s5
                                    nc.tensor.transpose(
                                        tps[:, s5 * 64 : (s5 + 1) * 64],
                                        q2d[eo : eo + 64, cb * 64 : (cb + 1) * 64],
                                        ident_b[eo : eo + 64, eo : eo + 64],
                                    )
                                # u-order store: col u = cb*64 + sl (dense)
                                nc.scalar.copy(
                                    dst_sb[
                                        eo : eo + 64,
                                        st * S + half * 512 : st * S + (half + 1) * 512,
                                    ],
                                    tps[:, :],
                                )

        qk_projection(qt_sb, d_q, d_wq, d_bq)
        qk_projection(kt_sb, d_k, d_wk, d_bk)

        # -------- V projection (j-major form, then 64x64 block transposes) ----
        with tc.tile_pool(name="pvt2d", bufs=1) as pvt2d:
            vt2d = pvt2d.tile([P, NT * S], F32, tag="vt2d")
            with tc.tile_pool(name="pxTv", bufs=1) as pxTv:
                xTv = pxTv.tile([P, NT * S], F32, tag="xTv")
                with (
                    tc.tile_pool(name="pxv", bufs=1) as pxv,
                    tc.tile_pool(name="ppsAv", bufs=2, space="PSUM") as ppsAv,
                ):
                    in_transpose(xTv, d_v, ppsAv, pxv, F32)
                with (
                    tc.tile_pool(name="pwv", bufs=18) as pwv,
                    tc.tile_pool(name="pbv", bufs=1) as pbv,
                    tc.tile_pool(name="ppsBv", bufs=2, space="PSUM") as ppsBv,
                ):
                    bv_row = pbv.tile([1, HD], F32, tag="bvrow")
                    nc.sync.dma_start(out=bv_row[:, :], in_=d_bv[:, :])
                    for jt3 in range(NT):
                        wts = []
                        for dt_ in range(NKT):
                            wt = pwv.tile([P, P], F32, tag="wv")
                            nc.sync.dma_start(
                                wt[:, :],
                                d_wv[dt_ * P : (dt_ + 1) * P, jt3 * P : (jt3 + 1) * P],
                            )
                            wts.append(wt)
                        pp = ppsBv.tile([P, S], F32, tag="ppv")
                        for ch in range(2):
                            sl = slice(ch * 512, (ch + 1) * 512)
                            for dt_ in range(NKT):
                                nc.tensor.matmul(
                                    pp[:, sl],
                                    lhsT=wts[dt_][:, :],
                                    rhs=xTv[:, dt_ * S + ch * 512 : dt_ * S + (ch + 1) * 512],
                                    start=(dt_ == 0),
                                    stop=False,
                                )
                            nc.tensor.matmul(
                                pp[:, sl],
                                lhsT=bv_row[0:1, jt3 * P : (jt3 + 1) * P],
                                rhs=ones_f[0:1, :],
                                start=False,
                                stop=True,
                            )
                        nc.scalar.copy(vt2d[:, jt3 * S : (jt3 + 1) * S], pp[:, :])
            # block transposes into v_sb: vt2d (d, s) blocks -> (sl, d)
            with tc.tile_pool(name="ppsTv", bufs=2, space="PSUM") as ppsTv:
                for h in range(H):
                    for half in range(2):
                        for par in range(2):
                            po = par * 64
                            tps = ppsTv.tile([64, 256], F32, tag="tpsv")
                            for c4 in range(4):
                                kt2 = half * 4 + c4
                                nc.tensor.transpose(
                                    tps[:, c4 * 64 : (c4 + 1) * 64],
                                    vt2d[po : po + 64, kt2 * S + 64 * h : kt2 * S + 64 * h + 64],
                                    ident_f[po : po + 64, po : po + 64],
                                )
                            src = tps[:, :].rearrange("p (c4 d) -> p c4 d", c4=4)
                            dst = v_sb[po : po + 64, :].rearrange(
                                "p (kt g) -> p kt g", kt=NT
                            )[:, half * 4 : (half + 1) * 4, h * G : h * G + DV]
                            nc.scalar.copy(dst, src)

        # ---------------- attention ----------------
        with ExitStack() as cctx:
            penp = cctx.enter_context(tc.tile_pool(name="pen", bufs=1))
            maskf_sb = penp.tile([P, NT * S], BF16)  # 0/1 mask (u_q part, u_k free)
            penT_sb = penp.tile([P, NT * S], BF16)  # penalty^T (u_k part, u_q free)
            with (
                tc.tile_pool(name="mstream", bufs=3) as ms,
                tc.tile_pool(name="psC0", bufs=2, space="PSUM") as psC0,
            ):
                # mask rows are loaded u-permuted (row t(uq) -> partition uq)
                mask_uq = d_mask[:, :].rearrange("(sl c) k -> c sl k", sl=64)
                for qt in range(NT):
                    m_t = ms.tile([P, S], I32, tag="mask")
                    for cpar in range(2):
                        nc.sync.dma_start(
                            m_t[cpar * 64 : (cpar + 1) * 64, :],
                            mask_uq[2 * qt + cpar],
                        )
                    # 0/1 mask cast to bf16, cols permuted t_k -> u_k
                    src = m_t[:, :].rearrange("p (sl c) -> p c sl", c=16)
                    dst = maskf_sb[:, qt * S : (qt + 1) * S].rearrange(
                        "p (c sl) -> p c sl", c=16
                    )
                    nc.vector.tensor_copy(dst, src)
                for kt in range(NT):
                    for half in range(2):
                        tp = psC0.tile([P, 512], BF16, tag="ptp")
                        for q4 in range(4):
                            qt = half * 4 + q4
                            nc.tensor.transpose(
                                tp[:, q4 * P : (q4 + 1) * P],
                                maskf_sb[:, qt * S + kt * P : qt * S + (kt + 1) * P],
                                ident_b,
                            )
                        # penalty^T = mask^T * 1920 - 1920  (0 kept, -1920 masked)
                        nc.vector.tensor_scalar(
                            out=penT_sb[
                                :, kt * S + half * 512 : kt * S + (half + 1) * 512
                            ],
                            in0=tp[:, :],
                            scalar1=1920.0,
                            scalar2=-1920.0,
                            op0=OP.mult,
                            op1=OP.add,
                        )

            hp_expT = cctx.enter_context(tc.tile_pool(name="expT", bufs=4))
            hp_au = cctx.enter_context(tc.tile_pool(name="au", bufs=4))
            hp_af = cctx.enter_context(tc.tile_pool(name="af", bufs=2))
            hp_small = cctx.enter_context(tc.tile_pool(name="hsmall", bufs=4))
            hp_rzb = cctx.enter_context(tc.tile_pool(name="rzb", bufs=2))
            psT = cctx.enter_context(tc.tile_pool(name="psT", bufs=2, space="PSUM"))
            psPV = cctx.enter_context(tc.tile_pool(name="psPV", bufs=2, space="PSUM"))
            psQ = cctx.enter_context(tc.tile_pool(name="psQ", bufs=2, space="PSUM"))

            # heads are processed in pairs (same jt tile, partition offsets 0/64)
            # so their K=64 score matmuls can run in different PE row groups.
            for hp in range(H // 2):
                jt = hp
                hs = (2 * hp, 2 * hp + 1)
                pvs = [psPV.tile([G, S], F32, tag="pv", name=f"pv{hp}_{e}") for e in range(2)]
                ets = None
                for kt in range(NT):
                    ets = [hp_expT.tile([P, S], F32, tag="expT", name=f"et{hp}_{kt}_{e}") for e in range(2)]
                    for c2 in range(2):
                        sl = slice(c2 * 512, (c2 + 1) * 512)
                        pss = [psT.tile([P, 512], F32, tag="scT", name=f"scT{hp}_{kt}_{c2}_{e}") for e in range(2)]
                        for e in range(2):
                            po = e * 64
                            nc.tensor.matmul(
                                pss[e][:, :],
                                lhsT=kt_sb[po : po + 64, jt * S + kt * P : jt * S + (kt + 1) * P],
                                rhs=qt_sb[po : po + 64, jt * S + c2 * 512 : jt * S + (c2 + 1) * 512],
                                start=True,
                                stop=False,
                            )
                        for e in range(2):
                            nc.tensor.matmul(
                                pss[e][:, :],
                                lhsT=ident_b,
                                rhs=penT_sb[:, kt * S + c2 * 512 : kt * S + (c2 + 1) * 512],
                                start=False,
                                stop=True,
                            )
                        for e in range(2):
                            nc.scalar.activation(
                                ets[e][:, sl], pss[e][:, :], AF.Exp, scale=1.0 / DK
                            )
                            nc.tensor.matmul(
                                pvs[e][:, sl],
                                lhsT=v_sb[:, kt * VW + hs[e] * G : kt * VW + (hs[e] + 1) * G],
                                rhs=ets[e][:, sl],
                                start=(kt == 0),
                                stop=(kt == NT - 1),
                            )
                rzt8s = []
                for e in range(2):
                    h = hs[e]
                    pv_ps = pvs[e]
                    # row G-1 of pv_ps is Z.  1/Z = exp(-ln(Z)) on ACT.
                    lnz = hp_small.tile([1, S], F32, tag="lnz")
                    nc.scalar.activation(lnz[:, :], pv_ps[G - 1 : G, :], AF.Ln)
                    lnzb = hp_rzb.tile([64, S], F32, tag="lnzb")
                    nc.gpsimd.partition_broadcast(lnzb[:, :], lnz[:, :])
                    rzb = hp_rzb.tile([64, S], F32, tag="rzb")
                    nc.scalar.activation(rzb[:, :], lnzb[:, :], AF.Exp, scale=-1.0)
                    # normalize rows 0..64 and scatter into out2n:
                    # psum[dv, u=(cb*64+sl)] ->
                    #   out2n_sb[(cb%2)*64 + dv, (cb//2)*S + 64h + sl]
                    src_v = pv_ps[0:64, :].rearrange(
                        "p (c4 pr sl) -> p pr c4 sl", c4=8, pr=2
                    )
                    rz_v = rzb[:, :].rearrange("p (c4 pr sl) -> p pr c4 sl", c4=8, pr=2)
                    for par in range(2):
                        dst = out2n_sb[par * 64 : (par + 1) * 64, :].rearrange(
                            "p (jt col) -> p jt col", jt=NT
                        )[:, :, 64 * h : 64 * h + 64]
                        nc.vector.tensor_tensor(
                            dst, src_v[:, par], rz_v[:, par], OP.mult
                        )
                    # 1/Z transposed to q-layout: (128 uq, 1) per q-tile, via PE
                    rzt_ps = psT.tile([P, 512], F32, tag="scT")
                    for qt in range(NT):
                        nc.tensor.transpose(
                            rzt_ps[:, qt * 64 : (qt + 1) * 64],
                            rzb[0:64, qt * P : (qt + 1) * P],
                            ident_f[0:64, 0:64],
                        )
                    rzt8 = hp_small.tile([P, NT], F32, tag="rzt8")
                    nc.scalar.copy(
                        rzt8[:, :],
                        rzt_ps[:, :].rearrange("p (q o) -> p q o", q=NT)[:, :, 0],
                    )
                    rzt8s.append(rzt8)

                # q path: scores (u_q, u_k) -> exp -> fused mask+normalize
                for qt in range(NT):
                    aus = [hp_au.tile([P, S], F32, tag="au", name=f"au{hp}_{qt}_{e}") for e in range(2)]
                    for c2 in range(2):
                        sl = slice(c2 * 512, (c2 + 1) * 512)
                        ps2s = [psQ.tile([P, 512], F32, tag="scQ", name=f"scQ{hp}_{qt}_{c2}_{e}") for e in range(2)]
                        for e in range(2):
                            po = e * 64
                            nc.tensor.matmul(
                                ps2s[e][:, :],
                                lhsT=qt_sb[po : po + 64, jt * S + qt * P : jt * S + (qt + 1) * P],
                                rhs=kt_sb[po : po + 64, jt * S + c2 * 512 : jt * S + (c2 + 1) * 512],
                                start=True,
                                stop=True,
                            )
                        for e in range(2):
                            nc.scalar.activation(
                                aus[e][:, sl], ps2s[e][:, :], AF.Exp, scale=1.0 / DK
                            )
                    for e in range(2):
                        h = hs[e]
                        # af = (au * 1/Z) * mask, with column permute u_k -> t_k
                        af = hp_af.tile([P, S], F32, tag="af")
                        src = aus[e][:, :].rearrange("p (c sl) -> p c sl", c=16)
                        msrc = maskf_sb[:, qt * S : (qt + 1) * S].rearrange(
                            "p (c sl) -> p c sl", c=16
                        )
                        dst = af[:, :].rearrange("p (sl c) -> p c sl", c=16)
                        nc.vector.scalar_tensor_tensor(
                            dst, src, rzt8s[e][:, qt : qt + 1], msrc, OP.mult, OP.mult
                        )
                        attn_uq = d_attn[h, :, :].rearrange("(sl c) k -> c sl k", sl=64)
                        for cpar in range(2):
                            nc.sync.dma_start(
                                attn_uq[2 * qt + cpar],
                                af[cpar * 64 : (cpar + 1) * 64, :],
                            )

        # ---------------- output projection ----------------
        with (
            tc.tile_pool(name="pwo", bufs=1) as pwo,
            tc.tile_pool(name="dstream", bufs=3) as ds_,
            tc.tile_pool(name="psD", bufs=2, space="PSUM") as psD,
        ):
            bo_bc = pwo.tile([P, DM], F32, tag="bobc")
            nc.gpsimd.dma_start(
                out=bo_bc[:, :],
                in_=bass.AP(tensor=d_bo, offset=0, ap=[[0, P], [1, DM]]),
            )
            wo_sb = pwo.tile([P, NKT * DM], F32)
            for jt2 in range(NKT):
                nc.sync.dma_start(
                    wo_sb[:, jt2 * DM : (jt2 + 1) * DM], d_wo[jt2 * P : (jt2 + 1) * P, :]
                )
            for qt in range(NT):
                for ec in range(2):
                    od_ps = psD.tile([P, 512], F32, tag="od")
                    for jt2 in range(NKT):
                        nc.tensor.matmul(
                            od_ps[:, :],
                            lhsT=out2n_sb[:, jt2 * S + qt * P : jt2 * S + (qt + 1) * P],
                            rhs=wo_sb[:, jt2 * DM + ec * 512 : jt2 * DM + (ec + 1) * 512],
                            start=(jt2 == 0),
                            stop=(jt2 == NKT - 1),
                        )
                    o_t = ds_.tile([P, 512], F32, tag="osb")
                    nc.vector.tensor_tensor(
                        o_t[:, :], od_ps[:, :], bo_bc[:, ec * 512 : (ec + 1) * 512], OP.add
                    )
                    nc.sync.dma_start(
                        d_out[qt * P : (qt + 1) * P, ec * 512 : (ec + 1) * 512], o_t[:, :]
                    )

    return nc


def get_nc(finalize=True):
    key = ("nc", finalize)
    if key not in _CACHE:
        nc = _build_nc()
        if finalize:
            nc.finalize()
        _CACHE[key] = nc
    return _CACHE[key]


def make_in_maps(inputs):
    q = np.asarray(inputs["query"], np.float32)
    k = np.asarray(inputs["key"], np.float32)
    v = np.asarray(inputs["value"], np.float32)
    m = np.asarray(inputs["mask"], np.int32)
    wq = np.asarray(inputs["Wq"], np.float32)
    wk = np.asarray(inputs["Wk"], np.float32)
    wv = np.asarray(inputs["Wv"], np.float32)
    wo = np.asarray(inputs["Wo"], np.float32)
    bq = np.asarray(inputs["bq"], np.float32).reshape(1, HD)
    bk = np.asarray(inputs["bk"], np.float32).reshape(1, HD)
    bv = np.asarray(inputs["bv"], np.float32).reshape(1, HD)
    bo = np.asarray(inputs["bo"], np.float32).reshape(1, DM)
    in_maps = []
    for c in range(N_CORES):
        in_maps.append(
            {
                "x_q": np.ascontiguousarray(q[c]),
                "x_k": np.ascontiguousarray(k[c]),
                "x_v": np.ascontiguousarray(v[c]),
                "mask": np.ascontiguousarray(m[c]),
                "Wq": wq,
                "Wk": wk,
                "Wv": wv,
                "Wo": wo,
                "bq": bq,
                "bk": bk,
                "bv": bv,
                "bo": bo,
            }
        )
    return in_maps


def kernel(**inputs):
    nc = get_nc(finalize=True)
    in_maps = make_in_maps(inputs)
    res = run_bass_kernel_spmd(nc, in_maps, core_ids=list(range(N_CORES)))
    out = np.stack([res.results[c]["out"] for c in range(N_CORES)], axis=0)
    attn = np.stack([res.results[c]["attn"] for c in range(N_CORES)], axis=0)
    return out, attn
